# revision 12
# baseline (speedup 1.0000x reference)
"""Trainium2 Bass kernel for nn_ClusterlingLayer (ragged_sequence).

Computes, for B=131072 fibers against K=64 clusters:
  x_dis[b,k] = ||x_b||^2 + ||w_k||^2 - 2 x_b.w_k
  dice[b,k]  = 1 - (2*inter + s)/(nF + nC + s)   (inter = ragged ROI histogram dot)
  q = rownorm( 1 / (1 + x_dis*dice) )
Returns (q, x_dis) like the reference.

Sharding: data-parallel over B across 8 NeuronCores (16384 fibers/core).
Host prep is limited to layout transforms (fiber sort by length, x transpose,
dtype casts, sentinel fold into rois), input norms, and K-side constants.
All B-proportional compute (matmul, per-fiber ROI histograms, dice math)
runs on device.

Device strategy per 128-fiber subtile (fibers globally sorted by length and
dealt round-robin so all 8 cores share one compile-time length profile; slots
are interleaved so each granule mixes short and long fibers):
 - per-fiber vocab histogram via fused custom DVE ops (HIST3F: 3 is_equal
   compares -> fresh partial; HIST2: 2 compares + accumulate), chain length
   sized to the subtile's max fiber length. A balanced subset of the longest
   subtiles runs on GPSIMD instead (tensor_scalar + STT chain) so DVE and
   GPSIMD finish together. Rois carry a sentinel bin (128) folded on host.
 - PE transposes the bf16 histogram -> [vocab, fiber] (identity matmul),
   ACT copies the 4-subtile block back to SBUF in one op, PE contracts with
   tbl2 = [1 - 2*histC^T | ones] plus an nC augment row so PSUM holds
   a = nF + nC - 2*inter and dens = nF + nC + s directly.
 - x_dis via PE in bf16: 4 accumulating (-2 w^T) d-chunks + a rank-3 augment
   (ones/xsq_hi/xsq_lo rows) folding in ||x||^2 near-exactly and ||w||^2.
 - elementwise entirely on GPSIMD: cden = dens + x_dis*a; q_un = dens/cden
   (divide); row-reduce; qf = q_un/rs. DVE stays pure-histogram.
 - DMA: inputs issue from the DVE HWDGE queue, outputs from SP, so an output
   DMA waiting on compute never blocks input prefetch (DMA waits hold the
   issuing SEQ). Outputs use a partition-major DRAM layout for 512B runs.
"""

import os
import sys

import numpy as np

for _p in ("/opt/trn_rl_repo", os.path.expanduser("~/.axon_site/_ro/trn_rl_repo")):
    if os.path.isdir(_p) and _p not in sys.path:
        sys.path.insert(0, _p)

import concourse.bass as bass
import concourse.mybir as mybir
import concourse.tile as tile
from concourse import bacc
from concourse.bass_utils import run_bass_kernel_spmd

import ml_dtypes


def _register_hist_ops():
    """Register fused histogram DVE ops (2-3 is_equal compares + accumulate
    per instruction) in the custom-DVE registry. Self-pins the uop shas."""
    from concourse import dve_ops
    from concourse.dve_spec import (
        Spec, Src0, Src1, C0, C1, C3, eq, _spill_c3_to_src1, lower,
        _has_src1 as has_src1,
    )

    if "HIST2_ANT" in dve_ops._SUB_OPCODE_FOR_NAME:
        return

    h2 = dve_ops.DveOp(
        "HIST2_ANT",
        Spec(
            body=eq(Src0, C0) + eq(Src0, C1) + Src1,
            reference=lambda in0, in1, s0, s1, imm2: (
                (in0 == s0) + (in0 == s1) + in1
            ).astype(np.float32),
        ),
        subdim=False,
        uops_sha={},
    )
    h3 = dve_ops.DveOp(
        "HIST3F_ANT",
        Spec(
            body=_spill_c3_to_src1(eq(Src0, C0) + eq(Src0, C1) + eq(Src0, C3)),
            reference=lambda in0, in1, s0, s1, imm2: (
                (in0 == s0) + (in0 == s1) + (in0 == in1.reshape(-1, 1)[:, :1])
            ).astype(np.float32),
        ),
        subdim=False,
        uops_sha={},
    )
    for op in (h2, h3):
        dve_ops.OPS.append(op)
        dve_ops.CUSTOM_DVE_SPECS[op.name] = op.spec
        dve_ops._SUB_OPCODE_FOR_NAME[op.name] = (
            max(dve_ops._SUB_OPCODE_FOR_NAME.values()) + 1
        )
    for op in (h2, h3):
        for ver in ("v3", "v4"):
            spec_c = dve_ops.DveOpSpec(
                name=op.name,
                opcode=dve_ops.get_dve_sub_opcode(op.name),
                uops=lower(op.spec, ver=ver),
                rd1_en=has_src1(op.spec),
            )
            op.uops_sha[ver] = spec_c.sha(ver)
    return

NCORES = 8
B, D, K, LF, LC = 131072, 512, 64, 24, 64
V = 128            # ROI vocab
BS = B // NCORES   # fibers per core
SUB = 128          # fibers per subtile (partition dim)
GRAN = 512         # fibers per granule
NGRAN = BS // GRAN
NSUB = GRAN // SUB
NSLOT = BS // SUB  # 128 subtile slots per core
SMOOTH = 1e-6
HB = 130           # histogram bins incl. sentinel 128 (+pad to even)
LFP = 32           # roi columns incl. sentinel padding (512B DMA runs)

f32 = mybir.dt.float32
bf16 = mybir.dt.bfloat16

bfdt = ml_dtypes.bfloat16

# engine-time model (ns) used to balance the DVE/GPSIMD histogram split
_DVE_OP_NS = 196.0
_POOL_CMP_NS = 275.0


def _dve_chain_ops(m):
    return 0 if m <= 0 else 1 + max(0, (m - 3 + 1) // 2)


def _plan_pool_slots(maxlens):
    """Pick which subtile slots run their histogram on GPSIMD so that
    DVE and GPSIMD engine-busy finish together."""
    # GPSIMD cannot read per-partition scalar operands (TensorScalarPtr is
    # DVE-only), so histogram chains cannot offload to Pool. Kept as a hook.
    return set()


def _build_nc(maxlens):
    """Build the per-core program. maxlens[t] = max fiber length in subtile
    slot t (shared across cores via the round-robin deal)."""
    _register_hist_ops()
    from concourse.dve_ops import OPS as _OPS
    _h2 = next(o for o in _OPS if o.name == "HIST2_ANT")
    _h3 = next(o for o in _OPS if o.name == "HIST3F_ANT")

    pool_slots = _plan_pool_slots(maxlens)

    nc = bacc.Bacc("TRN2", target_bir_lowering=False)

    xT = nc.dram_tensor("xT", [D, BS], bf16, kind="ExternalInput")
    aug3 = nc.dram_tensor("aug3", [3, BS], bf16, kind="ExternalInput")
    rl = nc.dram_tensor("rl", [SUB, NSLOT, LFP], f32, kind="ExternalInput")
    wT2 = nc.dram_tensor("wT2", [D, K], bf16, kind="ExternalInput")
    wsq3 = nc.dram_tensor("wsq3", [3, K], bf16, kind="ExternalInput")
    tbl2 = nc.dram_tensor("tbl2", [V, 2 * K], bf16, kind="ExternalInput")
    aug2 = nc.dram_tensor("aug2", [1, 2 * K], bf16, kind="ExternalInput")
    iotav = nc.dram_tensor("iotav", [HB], bf16, kind="ExternalInput")
    ident = nc.dram_tensor("ident", [SUB, SUB], bf16, kind="ExternalInput")

    # partition-major outputs: [p, g, s, k] for 512B contiguous runs
    q_out = nc.dram_tensor("q_out", [SUB, NGRAN, NSUB, K], bf16,
                           kind="ExternalOutput")
    xd_out = nc.dram_tensor("xd_out", [SUB, NGRAN, NSUB, K], bf16,
                            kind="ExternalOutput")

    xT_v = xT[:].rearrange("(c p) n -> p c n", p=SUB)          # [128, 4, BS]

    def bcast_row(dram_ap, n):
        # DMA-read AP replicating a DRAM row across n partitions
        return bass.AP(
            tensor=dram_ap.tensor,
            offset=dram_ap.offset,
            ap=[[0, n]] + dram_ap.ap,
        )

    with tile.TileContext(nc) as tc:
        with (
            tc.tile_pool(name="consts", bufs=1) as consts,
            tc.tile_pool(name="xin", bufs=5) as xin,
            tc.tile_pool(name="rin", bufs=8) as rin,
            tc.tile_pool(name="hist", bufs=20) as hist,
            tc.tile_pool(name="histT", bufs=4) as histT,
            tc.tile_pool(name="ew", bufs=10) as ew,
            tc.tile_pool(name="outs", bufs=8) as outs,
            tc.tile_pool(name="psx", bufs=3, space="PSUM") as psx,
            tc.tile_pool(name="psi", bufs=2, space="PSUM") as psi,
            tc.tile_pool(name="pst", bufs=3, space="PSUM") as pst,
        ):
            # ---- constants (loaded once, SP queue; c_iov first: the
            # histogram chains need it before anything else) ----
            c_iov = consts.tile([SUB, HB], bf16)
            nc.sync.dma_start(out=c_iov, in_=bcast_row(iotav[:], SUB))
            c_id = consts.tile([SUB, SUB], bf16)
            nc.sync.dma_start(out=c_id, in_=ident[:])
            c_wT = consts.tile([SUB, 4, K], bf16)
            nc.sync.dma_start(out=c_wT, in_=wT2[:].rearrange("(c p) k -> p c k", p=SUB))
            c_wsq3 = consts.tile([3, K], bf16)
            nc.sync.dma_start(out=c_wsq3, in_=wsq3[:])
            c_tbl2 = consts.tile([V, 2 * K], bf16)
            nc.sync.dma_start(out=c_tbl2, in_=tbl2[:])
            c_aug2 = consts.tile([1, 2 * K], bf16)
            nc.sync.dma_start(out=c_aug2, in_=aug2[:])
            c_ones = consts.tile([1, SUB], bf16)
            nc.vector.memset(c_ones, 1.0)
            c_aug3 = consts.tile([3, BS], bf16)
            nc.sync.dma_start(out=c_aug3, in_=aug3[:])

            pend1 = []  # granules awaiting reciprocal + qn
            pend2 = []  # granules awaiting reduce + final normalize + DMA out

            def emit_stage1():
                # one granule behind: rc never head-blocks the DVE queue
                g1, cden1, dv1, xd1 = pend1.pop(0)
                rc = ew.tile([SUB, NSUB, K], f32, tag="rc")
                nc.vector.reciprocal(out=rc, in_=cden1)
                qn = ew.tile([SUB, NSUB, K], f32, tag="qn")
                nc.gpsimd.tensor_tensor(
                    out=qn, in0=dv1, in1=rc, op=mybir.AluOpType.mult,
                )
                pend2.append((g1, qn, xd1))

            def emit_stage2():
                # two granules behind: reduce/rn wait on long-finished qn.
                # The row-sum runs on ACT via accum_out (copy output unused).
                g2, qn2, xd2 = pend2.pop(0)
                rs = ew.tile([SUB, NSUB], f32, tag="rs")
                qsc = ew.tile([SUB, NSUB, K], f32, tag="qsc")
                for s2 in range(NSUB):
                    nc.scalar.activation(
                        out=qsc[:, s2, :], in_=qn2[:, s2, :],
                        func=mybir.ActivationFunctionType.Copy,
                        accum_out=rs[:, s2:s2 + 1],
                    )
                rn = ew.tile([SUB, NSUB], f32, tag="rn")
                nc.vector.reciprocal(out=rn, in_=rs)
                qf = outs.tile([SUB, NSUB, K], bf16, tag="qf")
                rn_ap = rn[:]
                rn_b = bass.AP(
                    tensor=rn_ap.tensor, offset=rn_ap.offset,
                    ap=list(rn_ap.ap) + [[0, K]],
                )
                nc.gpsimd.tensor_tensor(
                    out=qf, in0=qn2, in1=rn_b, op=mybir.AluOpType.mult,
                )
                # outputs from the SP queue (only other outputs behind them)
                nc.sync.dma_start(out=q_out[:, g2, :, :], in_=qf[:])
                nc.sync.dma_start(
                    out=xd_out[:, g2, :, :],
                    in_=xd2[:].rearrange("p (t k) -> p t k", k=K))

            for g in range(NGRAN):
                t0 = g * NSUB  # first subtile slot of this granule

                # deferred stages of older granules first: their deps are
                # long-satisfied, so they never head-block any engine FIFO.
                if len(pend1) >= 2:
                    emit_stage1()
                if len(pend2) >= 2:
                    emit_stage2()

                # inputs from the ACT HWDGE queue (ACT copies ahead of them
                # complete promptly); outputs go to SP so a stalled output
                # never blocks input prefetch (DMA waits hold the SEQ).
                rt = rin.tile([SUB, NSUB, LFP], f32, tag="rt")
                nc.scalar.dma_start(out=rt, in_=rl[:, t0:t0 + NSUB, :])
                xt = xin.tile([SUB, 4, GRAN], bf16, tag="xt")
                nc.scalar.dma_start(out=xt, in_=xT_v[:, :, g * GRAN:(g + 1) * GRAN])

                psum_x = psx.tile([SUB, NSUB * K], f32, tag="px")
                psum_ad = psi.tile([SUB, NSUB, 2, K], f32, tag="pad")
                ptm = pst.tile([SUB, NSUB, SUB], bf16, tag="ptm")

                any_hist = any(maxlens[t0 + s] > 0 for s in range(NSUB))

                # x_dis matmuls first: PE work with no histogram dependency
                for s in range(NSUB):
                    for c in range(4):
                        nc.tensor.matmul(
                            psum_x[:, s * K:(s + 1) * K],
                            lhsT=xt[:, c, s * SUB:(s + 1) * SUB],
                            rhs=c_wT[:, c, :],
                            start=(c == 0), stop=False,
                        )
                    nc.tensor.matmul(
                        psum_x[:, s * K:(s + 1) * K],
                        lhsT=c_aug3[:, g * GRAN + s * SUB:g * GRAN + (s + 1) * SUB],
                        rhs=c_wsq3,
                        start=False, stop=True,
                    )

                for s in range(NSUB):
                    m = maxlens[t0 + s]
                    sc = lambda j: rt[:, s, j:j + 1]
                    # ---- per-fiber vocab histogram chain, sized to this
                    # subtile's max length; sentinel rois land in bin 128,
                    # excluded from the transpose.
                    if m > 0:
                        ha = hist.tile([SUB, HB], bf16, tag="ha")
                        hb = hist.tile([SUB, HB], bf16, tag="hb")
                        if (t0 + s) in pool_slots:
                            nc.gpsimd.tensor_scalar(
                                out=ha, in0=c_iov, scalar1=sc(0), scalar2=None,
                                op0=mybir.AluOpType.is_equal,
                            )
                            cur, nxt = ha, hb
                            for j in range(1, m):
                                nc.gpsimd.scalar_tensor_tensor(
                                    out=nxt, in0=c_iov, scalar=sc(j), in1=cur,
                                    op0=mybir.AluOpType.is_equal,
                                    op1=mybir.AluOpType.add,
                                )
                                cur, nxt = nxt, cur
                        else:
                            nc.vector._custom_dve(
                                _h3, out=ha, in0=c_iov, in1=sc(2),
                                s0=sc(0), s1=sc(1))
                            cur, nxt = ha, hb
                            for j0 in range(3, m, 2):
                                nc.vector._custom_dve(
                                    _h2, out=nxt, in0=c_iov, in1=cur,
                                    s0=sc(j0), s1=sc(j0 + 1))
                                cur, nxt = nxt, cur
                        nc.tensor.transpose(
                            out=ptm[:, s, :], in_=cur[:, 0:V], identity=c_id)

                xd = outs.tile([SUB, NSUB * K], bf16, tag="xd")
                nc.scalar.copy(out=xd, in_=psum_x)  # ACT: PSUM -> SBUF bf16

                if any_hist:
                    hTm = histT.tile([V, NSUB, SUB], bf16, tag="hTm")
                    nc.scalar.copy(out=hTm, in_=ptm)

                for s in range(NSUB):
                    m = maxlens[t0 + s]
                    # inter/dens: a = nF + nC - 2*inter, dens = nF + nC + s
                    if m > 0:
                        nc.tensor.matmul(
                            psum_ad[:, s, :, :], lhsT=hTm[:, s, :], rhs=c_tbl2,
                            start=True, stop=False,
                        )
                        nc.tensor.matmul(
                            psum_ad[:, s, :, :], lhsT=c_ones, rhs=c_aug2,
                            start=False, stop=True,
                        )
                    else:
                        nc.tensor.matmul(
                            psum_ad[:, s, :, :], lhsT=c_ones, rhs=c_aug2,
                            start=True, stop=True,
                        )

                # ---- elementwise on the full granule [128, 256], GPSIMD ----
                ad = ew.tile([SUB, NSUB, 2, K], f32, tag="ad")
                nc.scalar.copy(out=ad, in_=psum_ad)
                a_v = ad[:, :, 0, :]
                d_v = ad[:, :, 1, :]
                xd3 = xd[:].rearrange("p (t k) -> p t k", k=K)

                t_ = ew.tile([SUB, NSUB, K], f32, tag="t_")
                nc.gpsimd.tensor_tensor(
                    out=t_, in0=a_v, in1=xd3, op=mybir.AluOpType.mult,
                )
                cden = ew.tile([SUB, NSUB, K], f32, tag="cden")
                nc.gpsimd.tensor_tensor(
                    out=cden, in0=t_, in1=d_v, op=mybir.AluOpType.add,
                )
                pend1.append((g, cden, d_v, xd))

            while pend1:
                emit_stage1()
            while pend2:
                emit_stage2()

    nc.finalize()  # runs Bacc.compile(): wait-splitting, reg alloc, nop fusion
    return nc


_NC_CACHE = None
_NC_KEY = None
_LAST = None


def _get_nc(maxlens=None):
    global _NC_CACHE, _NC_KEY
    if maxlens is None:
        assert _NC_CACHE is not None
        return _NC_CACHE
    key = tuple(int(m) for m in maxlens)
    if _NC_CACHE is None or _NC_KEY != key:
        _NC_CACHE = _build_nc(key)
        _NC_KEY = key
    return _NC_CACHE


def kernel(x, weight, fiber_rois, fiber_lens, cluster_rois, cluster_lens):
    x = np.asarray(x, np.float32)
    weight = np.asarray(weight, np.float32)
    fiber_rois = np.asarray(fiber_rois, np.int32)
    fiber_lens = np.asarray(fiber_lens, np.int32)
    cluster_rois = np.asarray(cluster_rois, np.int32)
    cluster_lens = np.asarray(cluster_lens, np.int32)

    # K-side host prep (tiny): cluster histogram table, norms, constants
    mC = (np.arange(LC)[None, :] < cluster_lens[:, None])
    histC = np.zeros((K, V), np.float32)
    for k in range(K):
        histC[k] = np.bincount(cluster_rois[k][mC[k]], minlength=V).astype(np.float32)
    nC = cluster_lens.astype(np.float32)
    # tbl2: [V, 2K]; left block 1 - 2*histC^T (-> a), right block ones (-> dens)
    tbl2 = np.concatenate(
        [1.0 - 2.0 * histC.T, np.ones((V, K), np.float32)], axis=1
    ).astype(bfdt)
    # aug2: [1, 2K]; left nC, right nC + smooth
    aug2 = np.concatenate([nC, nC + SMOOTH])[None, :].astype(bfdt)
    wsq = (weight * weight).sum(1).astype(np.float32)       # [K]
    wsq3 = np.stack([wsq, np.ones(K, np.float32), np.ones(K, np.float32)])
    wsq3 = wsq3.astype(bfdt)                                # [3, K]
    iotav = np.arange(HB).astype(bfdt)
    ident = np.eye(SUB).astype(bfdt)
    wT2 = (-2.0 * weight.T).astype(bfdt)                    # [D, K]

    # fiber-side layout prep: sort by length, deal round-robin across cores
    # so every core shares one compile-time subtile length profile; then
    # interleave slots so each granule mixes all four length quartiles.
    order = np.argsort(fiber_lens, kind="stable")
    deal = order.reshape(NSLOT, NCORES, SUB)                # [slot, core, row]
    lens_sorted = fiber_lens[order].reshape(NSLOT, NCORES * SUB)
    maxlens_sorted = lens_sorted.max(axis=1).astype(np.int64)
    slot_order = np.empty(NSLOT, np.int64)
    nq = NSLOT // 4
    for g in range(NGRAN):
        slot_order[4 * g + 0] = g
        slot_order[4 * g + 1] = nq + g
        slot_order[4 * g + 2] = 2 * nq + g
        slot_order[4 * g + 3] = 3 * nq + (g * 13) % nq
    deal = deal[slot_order]
    maxlens = maxlens_sorted[slot_order]

    xsq = np.einsum("bd,bd->b", x, x).astype(np.float32)    # input norms (f32)
    xsq_hi = xsq.astype(bfdt)
    xsq_lo = (xsq - xsq_hi.astype(np.float32)).astype(bfdt)
    ones_b = np.ones(B, bfdt)
    x_bf = x.astype(bfdt)
    # rois with sentinel fold + padding columns
    rois_p = np.full((B, LFP), V, np.float32)
    rois_p[:, :LF] = np.where(np.arange(LF)[None, :] < fiber_lens[:, None],
                              fiber_rois, V).astype(np.float32)

    nc = _get_nc(maxlens)
    in_maps = []
    perms = []
    for ci in range(NCORES):
        perm = deal[:, ci, :].reshape(BS)
        perms.append(perm)
        # rl layout [p, slot, j]: fiber of slot t, partition p is perm[t*128+p]
        rl_c = rois_p[perm].reshape(NSLOT, SUB, LFP).transpose(1, 0, 2)
        in_maps.append({
            "xT": np.ascontiguousarray(x_bf[perm].T),
            "aug3": np.ascontiguousarray(
                np.stack([ones_b[perm], xsq_hi[perm], xsq_lo[perm]])),
            "rl": np.ascontiguousarray(rl_c),
            "wT2": wT2,
            "wsq3": wsq3,
            "tbl2": tbl2,
            "aug2": aug2,
            "iotav": iotav,
            "ident": ident,
        })

    res = run_bass_kernel_spmd(nc, in_maps, core_ids=list(range(NCORES)))
    global _LAST
    _LAST = res
    q = np.empty((B, K), np.float32)
    xd = np.empty((B, K), np.float32)
    for ci in range(NCORES):
        # outputs are [p, g, s, k]; fiber row of slot t=4g+s, partition p
        # is perm[t*128 + p]
        qo = res.results[ci]["q_out"].astype(np.float32)
        xo = res.results[ci]["xd_out"].astype(np.float32)
        q[perms[ci]] = qo.reshape(SUB, NSLOT, K).transpose(1, 0, 2).reshape(BS, K)
        xd[perms[ci]] = xo.reshape(SUB, NSLOT, K).transpose(1, 0, 2).reshape(BS, K)
    return (q, xd)


# revision 19
# speedup vs baseline: 1.1140x; 1.1140x over previous
"""Trainium2 Bass kernel for nn_ClusterlingLayer (ragged_sequence).

Computes, for B=131072 fibers against K=64 clusters:
  x_dis[b,k] = ||x_b||^2 + ||w_k||^2 - 2 x_b.w_k
  dice[b,k]  = 1 - (2*inter + s)/(nF + nC + s)   (inter = ragged ROI histogram dot)
  q = rownorm( 1 / (1 + x_dis*dice) )
Returns (q, x_dis) like the reference.

Sharding: data-parallel over B across 8 NeuronCores (16384 fibers/core).
Host prep is limited to layout transforms (fiber sort by length, x transpose,
dtype casts, sentinel fold into rois), input norms, and K-side constants.
All B-proportional compute (matmul, per-fiber ROI histograms, dice math)
runs on device.

Device strategy per 128-fiber subtile (fibers globally sorted by length and
dealt round-robin so all 8 cores share one compile-time length profile; slots
are interleaved so each granule mixes short and long fibers):
 - per-fiber vocab histogram via fused custom DVE ops (HIST3F: 3 is_equal
   compares -> fresh partial; HIST2: 2 compares + accumulate), chain length
   sized to the subtile's max fiber length. A balanced subset of the longest
   subtiles runs on GPSIMD instead (tensor_scalar + STT chain) so DVE and
   GPSIMD finish together. Rois carry a sentinel bin (128) folded on host.
 - PE transposes the bf16 histogram -> [vocab, fiber] (identity matmul),
   ACT copies the 4-subtile block back to SBUF in one op, PE contracts with
   tbl2 = [1 - 2*histC^T | ones] plus an nC augment row so PSUM holds
   a = nF + nC - 2*inter and dens = nF + nC + s directly.
 - x_dis via PE in bf16: 4 accumulating (-2 w^T) d-chunks + a rank-3 augment
   (ones/xsq_hi/xsq_lo rows) folding in ||x||^2 near-exactly and ||w||^2.
 - elementwise entirely on GPSIMD: cden = dens + x_dis*a; q_un = dens/cden
   (divide); row-reduce; qf = q_un/rs. DVE stays pure-histogram.
 - DMA: inputs issue from the DVE HWDGE queue, outputs from SP, so an output
   DMA waiting on compute never blocks input prefetch (DMA waits hold the
   issuing SEQ). Outputs use a partition-major DRAM layout for 512B runs.
"""

import os
import sys

import numpy as np

for _p in ("/opt/trn_rl_repo", os.path.expanduser("~/.axon_site/_ro/trn_rl_repo")):
    if os.path.isdir(_p) and _p not in sys.path:
        sys.path.insert(0, _p)

import concourse.bass as bass
import concourse.mybir as mybir
import concourse.tile as tile
from concourse import bacc
from concourse.bass_utils import run_bass_kernel_spmd

import ml_dtypes


def _register_hist_ops():
    """Register fused histogram DVE ops (2-3 is_equal compares + accumulate
    per instruction) in the custom-DVE registry. Self-pins the uop shas."""
    from concourse import dve_ops
    from concourse.dve_spec import (
        Spec, Src0, Src1, C0, C1, C3, eq, _spill_c3_to_src1, lower,
        _has_src1 as has_src1,
    )

    if "HIST2_ANT" in dve_ops._SUB_OPCODE_FOR_NAME:
        return

    h2 = dve_ops.DveOp(
        "HIST2_ANT",
        Spec(
            body=eq(Src0, C0) + eq(Src0, C1) + Src1,
            reference=lambda in0, in1, s0, s1, imm2: (
                (in0 == s0) + (in0 == s1) + in1
            ).astype(np.float32),
        ),
        subdim=False,
        uops_sha={},
    )
    h3 = dve_ops.DveOp(
        "HIST3F_ANT",
        Spec(
            body=_spill_c3_to_src1(eq(Src0, C0) + eq(Src0, C1) + eq(Src0, C3)),
            reference=lambda in0, in1, s0, s1, imm2: (
                (in0 == s0) + (in0 == s1) + (in0 == in1.reshape(-1, 1)[:, :1])
            ).astype(np.float32),
        ),
        subdim=False,
        uops_sha={},
    )
    for op in (h2, h3):
        dve_ops.OPS.append(op)
        dve_ops.CUSTOM_DVE_SPECS[op.name] = op.spec
        dve_ops._SUB_OPCODE_FOR_NAME[op.name] = (
            max(dve_ops._SUB_OPCODE_FOR_NAME.values()) + 1
        )
    for op in (h2, h3):
        for ver in ("v3", "v4"):
            spec_c = dve_ops.DveOpSpec(
                name=op.name,
                opcode=dve_ops.get_dve_sub_opcode(op.name),
                uops=lower(op.spec, ver=ver),
                rd1_en=has_src1(op.spec),
            )
            op.uops_sha[ver] = spec_c.sha(ver)
    return

NCORES = 8
B, D, K, LF, LC = 131072, 512, 64, 24, 64
V = 128            # ROI vocab
BS = B // NCORES   # fibers per core
SUB = 128          # fibers per subtile (partition dim)
GRAN = 512         # fibers per granule
NGRAN = BS // GRAN
NSUB = GRAN // SUB
NSLOT = BS // SUB  # 128 subtile slots per core
SMOOTH = 1e-6
HB = 130           # histogram bins incl. sentinel 128 (+pad to even)
LFP = 32           # roi columns incl. sentinel padding (512B DMA runs)

f32 = mybir.dt.float32
bf16 = mybir.dt.bfloat16

bfdt = ml_dtypes.bfloat16

# engine-time model (ns) used to balance the DVE/GPSIMD histogram split
_DVE_OP_NS = 196.0
_POOL_CMP_NS = 275.0


def _dve_chain_ops(m):
    return 0 if m <= 0 else 1 + max(0, (m - 3 + 1) // 2)


def _plan_pool_slots(maxlens):
    """Pick which subtile slots run their histogram on GPSIMD so that
    DVE and GPSIMD engine-busy finish together."""
    # GPSIMD cannot read per-partition scalar operands (TensorScalarPtr is
    # DVE-only), so histogram chains cannot offload to Pool. Kept as a hook.
    return set()


def _build_nc(maxlens, stage_top=False, d1=6, d2=4, xd_early=True):
    """Build the per-core program. maxlens[t] = max fiber length in subtile
    slot t (shared across cores via the round-robin deal)."""
    _register_hist_ops()
    from concourse.dve_ops import OPS as _OPS
    _h2 = next(o for o in _OPS if o.name == "HIST2_ANT")
    _h3 = next(o for o in _OPS if o.name == "HIST3F_ANT")

    pool_slots = _plan_pool_slots(maxlens)

    nc = bacc.Bacc("TRN2", target_bir_lowering=False)

    xT = nc.dram_tensor("xT", [D, BS], bf16, kind="ExternalInput")
    aug3 = nc.dram_tensor("aug3", [3, BS], bf16, kind="ExternalInput")
    rl = nc.dram_tensor("rl", [SUB, NSLOT, LFP], f32, kind="ExternalInput")
    wT2 = nc.dram_tensor("wT2", [D, K], bf16, kind="ExternalInput")
    wsq3 = nc.dram_tensor("wsq3", [3, K], bf16, kind="ExternalInput")
    tbl2 = nc.dram_tensor("tbl2", [V, 2 * K], bf16, kind="ExternalInput")
    aug2 = nc.dram_tensor("aug2", [1, 2 * K], bf16, kind="ExternalInput")
    iotav = nc.dram_tensor("iotav", [HB], bf16, kind="ExternalInput")
    ident = nc.dram_tensor("ident", [SUB, SUB], bf16, kind="ExternalInput")

    # partition-major outputs: [p, g, s, k] for 512B contiguous runs
    q_out = nc.dram_tensor("q_out", [SUB, NGRAN, NSUB, K], bf16,
                           kind="ExternalOutput")
    xd_out = nc.dram_tensor("xd_out", [SUB, NGRAN, NSUB, K], bf16,
                            kind="ExternalOutput")

    xT_v = xT[:].rearrange("(c p) n -> p c n", p=SUB)          # [128, 4, BS]

    def bcast_row(dram_ap, n):
        # DMA-read AP replicating a DRAM row across n partitions
        return bass.AP(
            tensor=dram_ap.tensor,
            offset=dram_ap.offset,
            ap=[[0, n]] + dram_ap.ap,
        )

    with tile.TileContext(nc) as tc:
        with (
            tc.tile_pool(name="consts", bufs=1) as consts,
            tc.tile_pool(name="xin", bufs=5) as xin,
            tc.tile_pool(name="rin", bufs=8) as rin,
            tc.tile_pool(name="hist", bufs=20) as hist,
            tc.tile_pool(name="histT", bufs=4) as histT,
            tc.tile_pool(name="ew_ad", bufs=d1 + 3) as ew_ad,
            tc.tile_pool(name="ew_cd", bufs=d1 + 3) as ew_cd,
            tc.tile_pool(name="ew_sh", bufs=4) as ew_sh,
            tc.tile_pool(name="ew_qn", bufs=d2 + 3) as ew_qn,
            tc.tile_pool(name="ew_xd", bufs=d1 + d2 + 3) as ew_xd,
            tc.tile_pool(name="outs", bufs=4) as outs,
            tc.tile_pool(name="psx", bufs=3, space="PSUM") as psx,
            tc.tile_pool(name="psi", bufs=2, space="PSUM") as psi,
            tc.tile_pool(name="pst", bufs=3, space="PSUM") as pst,
        ):
            # ---- constants (loaded once, SP queue; c_iov first: the
            # histogram chains need it before anything else) ----
            c_iov = consts.tile([SUB, HB], bf16)
            nc.sync.dma_start(out=c_iov, in_=bcast_row(iotav[:], SUB))
            c_id = consts.tile([SUB, SUB], bf16)
            nc.sync.dma_start(out=c_id, in_=ident[:])
            c_wT = consts.tile([SUB, 4, K], bf16)
            nc.sync.dma_start(out=c_wT, in_=wT2[:].rearrange("(c p) k -> p c k", p=SUB))
            c_wsq3 = consts.tile([3, K], bf16)
            nc.sync.dma_start(out=c_wsq3, in_=wsq3[:])
            c_tbl2 = consts.tile([V, 2 * K], bf16)
            nc.sync.dma_start(out=c_tbl2, in_=tbl2[:])
            c_aug2 = consts.tile([1, 2 * K], bf16)
            nc.sync.dma_start(out=c_aug2, in_=aug2[:])
            c_ones = consts.tile([1, SUB], bf16)
            nc.vector.memset(c_ones, 1.0)
            c_aug3 = consts.tile([3, BS], bf16)
            nc.sync.dma_start(out=c_aug3, in_=aug3[:])

            pend1 = []  # granules awaiting reciprocal + qn
            pend2 = []  # granules awaiting reduce + final normalize + DMA out

            def emit_stage1():
                # one granule behind: rc never head-blocks the DVE queue
                g1, cden1, dv1, xd1 = pend1.pop(0)
                rc = ew_sh.tile([SUB, NSUB, K], f32, tag="rc")
                nc.vector.reciprocal(out=rc, in_=cden1)
                qn = ew_qn.tile([SUB, NSUB, K], f32, tag="qn")
                nc.gpsimd.tensor_tensor(
                    out=qn, in0=dv1, in1=rc, op=mybir.AluOpType.mult,
                )
                pend2.append((g1, qn, xd1))

            def emit_stage2():
                # two granules behind: reduce/rn wait on long-finished qn.
                # The row-sum runs on ACT via accum_out (copy output unused).
                g2, qn2, xd2 = pend2.pop(0)
                rs = ew_sh.tile([SUB, NSUB], f32, tag="rs")
                qsc = ew_sh.tile([SUB, NSUB, K], f32, tag="qsc")
                for s2 in range(NSUB):
                    nc.scalar.activation(
                        out=qsc[:, s2, :], in_=qn2[:, s2, :],
                        func=mybir.ActivationFunctionType.Copy,
                        accum_out=rs[:, s2:s2 + 1],
                    )
                rn = ew_sh.tile([SUB, NSUB], f32, tag="rn")
                nc.vector.reciprocal(out=rn, in_=rs)
                qf = outs.tile([SUB, NSUB, K], bf16, tag="qf")
                rn_ap = rn[:]
                rn_b = bass.AP(
                    tensor=rn_ap.tensor, offset=rn_ap.offset,
                    ap=list(rn_ap.ap) + [[0, K]],
                )
                nc.gpsimd.tensor_tensor(
                    out=qf, in0=qn2, in1=rn_b, op=mybir.AluOpType.mult,
                )
                # outputs from the SP queue (only other outputs behind them)
                nc.sync.dma_start(out=q_out[:, g2, :, :], in_=qf[:])
                nc.sync.dma_start(
                    out=xd_out[:, g2, :, :],
                    in_=xd2[:].rearrange("p (t k) -> p t k", k=K))

            for g in range(NGRAN):
                t0 = g * NSUB  # first subtile slot of this granule

                # deferred stages of older granules first: their deps are
                # long-satisfied, so they never head-block any engine FIFO.
                if stage_top:
                    if len(pend1) >= d1:
                        emit_stage1()
                    if len(pend2) >= d2:
                        emit_stage2()

                # inputs from the ACT HWDGE queue (ACT copies ahead of them
                # complete promptly); outputs go to SP so a stalled output
                # never blocks input prefetch (DMA waits hold the SEQ).
                rt = rin.tile([SUB, NSUB, LFP], f32, tag="rt")
                nc.scalar.dma_start(out=rt, in_=rl[:, t0:t0 + NSUB, :])
                xt = xin.tile([SUB, 4, GRAN], bf16, tag="xt")
                nc.scalar.dma_start(out=xt, in_=xT_v[:, :, g * GRAN:(g + 1) * GRAN])

                psum_x = psx.tile([SUB, NSUB * K], f32, tag="px")
                psum_ad = psi.tile([SUB, NSUB, 2, K], f32, tag="pad")
                ptm = pst.tile([SUB, NSUB, SUB], bf16, tag="ptm")

                any_hist = any(maxlens[t0 + s] > 0 for s in range(NSUB))

                # x_dis matmuls first: PE work with no histogram dependency
                for s in range(NSUB):
                    for c in range(4):
                        nc.tensor.matmul(
                            psum_x[:, s * K:(s + 1) * K],
                            lhsT=xt[:, c, s * SUB:(s + 1) * SUB],
                            rhs=c_wT[:, c, :],
                            start=(c == 0), stop=False,
                        )
                    nc.tensor.matmul(
                        psum_x[:, s * K:(s + 1) * K],
                        lhsT=c_aug3[:, g * GRAN + s * SUB:g * GRAN + (s + 1) * SUB],
                        rhs=c_wsq3,
                        start=False, stop=True,
                    )

                for s in range(NSUB):
                    m = maxlens[t0 + s]
                    sc = lambda j: rt[:, s, j:j + 1]
                    # ---- per-fiber vocab histogram chain, sized to this
                    # subtile's max length; sentinel rois land in bin 128,
                    # excluded from the transpose.
                    if m > 0:
                        ha = hist.tile([SUB, HB], bf16, tag="ha")
                        hb = hist.tile([SUB, HB], bf16, tag="hb")
                        if (t0 + s) in pool_slots:
                            nc.gpsimd.tensor_scalar(
                                out=ha, in0=c_iov, scalar1=sc(0), scalar2=None,
                                op0=mybir.AluOpType.is_equal,
                            )
                            cur, nxt = ha, hb
                            for j in range(1, m):
                                nc.gpsimd.scalar_tensor_tensor(
                                    out=nxt, in0=c_iov, scalar=sc(j), in1=cur,
                                    op0=mybir.AluOpType.is_equal,
                                    op1=mybir.AluOpType.add,
                                )
                                cur, nxt = nxt, cur
                        else:
                            nc.vector._custom_dve(
                                _h3, out=ha, in0=c_iov, in1=sc(2),
                                s0=sc(0), s1=sc(1))
                            cur, nxt = ha, hb
                            for j0 in range(3, m, 2):
                                nc.vector._custom_dve(
                                    _h2, out=nxt, in0=c_iov, in1=cur,
                                    s0=sc(j0), s1=sc(j0 + 1))
                                cur, nxt = nxt, cur
                        nc.tensor.transpose(
                            out=ptm[:, s, :], in_=cur[:, 0:V], identity=c_id)

                xd = ew_xd.tile([SUB, NSUB * K], bf16, tag="xd")
                if xd_early:
                    nc.scalar.copy(out=xd, in_=psum_x)  # ACT PSUM->SBUF bf16

                if any_hist:
                    hTm = histT.tile([V, NSUB, SUB], bf16, tag="hTm")
                    nc.scalar.copy(out=hTm, in_=ptm)

                for s in range(NSUB):
                    m = maxlens[t0 + s]
                    # inter/dens: a = nF + nC - 2*inter, dens = nF + nC + s
                    if m > 0:
                        nc.tensor.matmul(
                            psum_ad[:, s, :, :], lhsT=hTm[:, s, :], rhs=c_tbl2,
                            start=True, stop=False,
                        )
                        nc.tensor.matmul(
                            psum_ad[:, s, :, :], lhsT=c_ones, rhs=c_aug2,
                            start=False, stop=True,
                        )
                    else:
                        nc.tensor.matmul(
                            psum_ad[:, s, :, :], lhsT=c_ones, rhs=c_aug2,
                            start=True, stop=True,
                        )

                # ---- elementwise on the full granule [128, 256], GPSIMD ----
                if not xd_early:
                    nc.scalar.copy(out=xd, in_=psum_x)  # ACT PSUM->SBUF bf16
                ad = ew_ad.tile([SUB, NSUB, 2, K], f32, tag="ad")
                nc.scalar.copy(out=ad, in_=psum_ad)
                a_v = ad[:, :, 0, :]
                d_v = ad[:, :, 1, :]
                xd3 = xd[:].rearrange("p (t k) -> p t k", k=K)

                t_ = ew_sh.tile([SUB, NSUB, K], f32, tag="t_")
                nc.gpsimd.tensor_tensor(
                    out=t_, in0=a_v, in1=xd3, op=mybir.AluOpType.mult,
                )
                cden = ew_cd.tile([SUB, NSUB, K], f32, tag="cden")
                nc.gpsimd.tensor_tensor(
                    out=cden, in0=t_, in1=d_v, op=mybir.AluOpType.add,
                )
                pend1.append((g, cden, d_v, xd))
                if not stage_top:
                    if len(pend1) > d1:
                        emit_stage1()
                    if len(pend2) > d2:
                        emit_stage2()

            # interleaved drain: stage2 work overlaps the remaining recips
            while pend1 or pend2:
                if pend1:
                    emit_stage1()
                if pend2:
                    emit_stage2()

    nc.finalize()  # runs Bacc.compile(): wait-splitting, reg alloc, nop fusion
    return nc


_NC_CACHE = None
_NC_KEY = None
_LAST = None


def _get_nc(maxlens=None, **opts):
    global _NC_CACHE, _NC_KEY
    if maxlens is None:
        assert _NC_CACHE is not None
        return _NC_CACHE
    key = (tuple(int(m) for m in maxlens), tuple(sorted(opts.items())))
    if _NC_CACHE is None or _NC_KEY != key:
        _NC_CACHE = _build_nc(tuple(int(m) for m in maxlens), **opts)
        _NC_KEY = key
    return _NC_CACHE


def kernel(x, weight, fiber_rois, fiber_lens, cluster_rois, cluster_lens):
    x = np.asarray(x, np.float32)
    weight = np.asarray(weight, np.float32)
    fiber_rois = np.asarray(fiber_rois, np.int32)
    fiber_lens = np.asarray(fiber_lens, np.int32)
    cluster_rois = np.asarray(cluster_rois, np.int32)
    cluster_lens = np.asarray(cluster_lens, np.int32)

    # K-side host prep (tiny): cluster histogram table, norms, constants
    mC = (np.arange(LC)[None, :] < cluster_lens[:, None])
    histC = np.zeros((K, V), np.float32)
    for k in range(K):
        histC[k] = np.bincount(cluster_rois[k][mC[k]], minlength=V).astype(np.float32)
    nC = cluster_lens.astype(np.float32)
    # tbl2: [V, 2K]; left block 1 - 2*histC^T (-> a), right block ones (-> dens)
    tbl2 = np.concatenate(
        [1.0 - 2.0 * histC.T, np.ones((V, K), np.float32)], axis=1
    ).astype(bfdt)
    # aug2: [1, 2K]; left nC, right nC + smooth
    aug2 = np.concatenate([nC, nC + SMOOTH])[None, :].astype(bfdt)
    wsq = (weight * weight).sum(1).astype(np.float32)       # [K]
    wsq3 = np.stack([wsq, np.ones(K, np.float32), np.ones(K, np.float32)])
    wsq3 = wsq3.astype(bfdt)                                # [3, K]
    iotav = np.arange(HB).astype(bfdt)
    ident = np.eye(SUB).astype(bfdt)
    wT2 = (-2.0 * weight.T).astype(bfdt)                    # [D, K]

    # fiber-side layout prep: sort by length, deal round-robin across cores
    # so every core shares one compile-time subtile length profile; then
    # interleave slots so each granule mixes all four length quartiles.
    order = np.argsort(fiber_lens, kind="stable")
    deal = order.reshape(NSLOT, NCORES, SUB)                # [slot, core, row]
    lens_sorted = fiber_lens[order].reshape(NSLOT, NCORES * SUB)
    maxlens_sorted = lens_sorted.max(axis=1).astype(np.int64)
    slot_order = np.empty(NSLOT, np.int64)
    nq = NSLOT // 4
    for g in range(NGRAN):
        slot_order[4 * g + 0] = g
        slot_order[4 * g + 1] = nq + g
        slot_order[4 * g + 2] = 2 * nq + g
        slot_order[4 * g + 3] = 3 * nq + (g * 13) % nq
    deal = deal[slot_order]
    maxlens = maxlens_sorted[slot_order]

    xsq = np.einsum("bd,bd->b", x, x).astype(np.float32)    # input norms (f32)
    xsq_hi = xsq.astype(bfdt)
    xsq_lo = (xsq - xsq_hi.astype(np.float32)).astype(bfdt)
    ones_b = np.ones(B, bfdt)
    x_bf = x.astype(bfdt)
    # rois with sentinel fold + padding columns
    rois_p = np.full((B, LFP), V, np.float32)
    rois_p[:, :LF] = np.where(np.arange(LF)[None, :] < fiber_lens[:, None],
                              fiber_rois, V).astype(np.float32)

    nc = _get_nc(maxlens)
    in_maps = []
    perms = []
    for ci in range(NCORES):
        perm = deal[:, ci, :].reshape(BS)
        perms.append(perm)
        # rl layout [p, slot, j]: fiber of slot t, partition p is perm[t*128+p]
        rl_c = rois_p[perm].reshape(NSLOT, SUB, LFP).transpose(1, 0, 2)
        in_maps.append({
            "xT": np.ascontiguousarray(x_bf[perm].T),
            "aug3": np.ascontiguousarray(
                np.stack([ones_b[perm], xsq_hi[perm], xsq_lo[perm]])),
            "rl": np.ascontiguousarray(rl_c),
            "wT2": wT2,
            "wsq3": wsq3,
            "tbl2": tbl2,
            "aug2": aug2,
            "iotav": iotav,
            "ident": ident,
        })

    res = run_bass_kernel_spmd(nc, in_maps, core_ids=list(range(NCORES)))
    global _LAST
    _LAST = res
    q = np.empty((B, K), np.float32)
    xd = np.empty((B, K), np.float32)
    for ci in range(NCORES):
        # outputs are [p, g, s, k]; fiber row of slot t=4g+s, partition p
        # is perm[t*128 + p]
        qo = res.results[ci]["q_out"].astype(np.float32)
        xo = res.results[ci]["xd_out"].astype(np.float32)
        q[perms[ci]] = qo.reshape(SUB, NSLOT, K).transpose(1, 0, 2).reshape(BS, K)
        xd[perms[ci]] = xo.reshape(SUB, NSLOT, K).transpose(1, 0, 2).reshape(BS, K)
    return (q, xd)


# revision 28
# speedup vs baseline: 1.1626x; 1.0437x over previous
"""Trainium2 Bass kernel for nn_ClusterlingLayer (ragged_sequence).

Computes, for B=131072 fibers against K=64 clusters:
  x_dis[b,k] = ||x_b||^2 + ||w_k||^2 - 2 x_b.w_k
  dice[b,k]  = 1 - (2*inter + s)/(nF + nC + s)   (inter = ragged ROI histogram dot)
  q = rownorm( 1 / (1 + x_dis*dice) )
Returns (q, x_dis) like the reference.

Sharding: data-parallel over B across 8 NeuronCores (16384 fibers/core).
Host prep is limited to layout transforms (fiber sort by length, x transpose,
dtype casts, sentinel fold into rois), input norms, and K-side constants.
All B-proportional compute (matmul, per-fiber ROI histograms, dice math)
runs on device.

Device strategy per 128-fiber subtile (fibers globally sorted by length and
dealt round-robin so all 8 cores share one compile-time length profile; slots
are interleaved so each granule mixes short and long fibers):
 - per-fiber vocab histogram via fused custom DVE ops (HIST3F: 3 is_equal
   compares -> fresh partial; HIST2: 2 compares + accumulate), chain length
   sized to the subtile's max fiber length. A balanced subset of the longest
   subtiles runs on GPSIMD instead (tensor_scalar + STT chain) so DVE and
   GPSIMD finish together. Rois carry a sentinel bin (128) folded on host.
 - PE transposes the bf16 histogram -> [vocab, fiber] (identity matmul),
   ACT copies the 4-subtile block back to SBUF in one op, PE contracts with
   tbl2 = [1 - 2*histC^T | ones] plus an nC augment row so PSUM holds
   a = nF + nC - 2*inter and dens = nF + nC + s directly.
 - x_dis via PE in bf16: 4 accumulating (-2 w^T) d-chunks + a rank-3 augment
   (ones/xsq_hi/xsq_lo rows) folding in ||x||^2 near-exactly and ||w||^2.
 - elementwise entirely on GPSIMD: cden = dens + x_dis*a; q_un = dens/cden
   (divide); row-reduce; qf = q_un/rs. DVE stays pure-histogram.
 - DMA: inputs issue from the DVE HWDGE queue, outputs from SP, so an output
   DMA waiting on compute never blocks input prefetch (DMA waits hold the
   issuing SEQ). Outputs use a partition-major DRAM layout for 512B runs.
"""

import os
import sys

import numpy as np

for _p in ("/opt/trn_rl_repo", os.path.expanduser("~/.axon_site/_ro/trn_rl_repo")):
    if os.path.isdir(_p) and _p not in sys.path:
        sys.path.insert(0, _p)

import concourse.bass as bass
import concourse.mybir as mybir
import concourse.tile as tile
from concourse import bacc
from concourse.bass_utils import run_bass_kernel_spmd

import ml_dtypes


def _register_hist_ops():
    """Register fused histogram DVE ops (2-3 is_equal compares + accumulate
    per instruction) in the custom-DVE registry. Self-pins the uop shas."""
    from concourse import dve_ops
    from concourse.dve_spec import (
        Spec, Src0, Src1, C0, C1, C3, eq, _spill_c3_to_src1, lower,
        _has_src1 as has_src1,
    )

    if "HIST2_ANT" in dve_ops._SUB_OPCODE_FOR_NAME:
        return

    h2 = dve_ops.DveOp(
        "HIST2_ANT",
        Spec(
            body=eq(Src0, C0) + eq(Src0, C1) + Src1,
            reference=lambda in0, in1, s0, s1, imm2: (
                (in0 == s0) + (in0 == s1) + in1
            ).astype(np.float32),
        ),
        subdim=False,
        uops_sha={},
    )
    h3 = dve_ops.DveOp(
        "HIST3F_ANT",
        Spec(
            body=_spill_c3_to_src1(eq(Src0, C0) + eq(Src0, C1) + eq(Src0, C3)),
            reference=lambda in0, in1, s0, s1, imm2: (
                (in0 == s0) + (in0 == s1) + (in0 == in1.reshape(-1, 1)[:, :1])
            ).astype(np.float32),
        ),
        subdim=False,
        uops_sha={},
    )
    for op in (h2, h3):
        dve_ops.OPS.append(op)
        dve_ops.CUSTOM_DVE_SPECS[op.name] = op.spec
        dve_ops._SUB_OPCODE_FOR_NAME[op.name] = (
            max(dve_ops._SUB_OPCODE_FOR_NAME.values()) + 1
        )
    for op in (h2, h3):
        for ver in ("v3", "v4"):
            spec_c = dve_ops.DveOpSpec(
                name=op.name,
                opcode=dve_ops.get_dve_sub_opcode(op.name),
                uops=lower(op.spec, ver=ver),
                rd1_en=has_src1(op.spec),
            )
            op.uops_sha[ver] = spec_c.sha(ver)
    return

NCORES = 8
B, D, K, LF, LC = 131072, 512, 64, 24, 64
V = 128            # ROI vocab
BS = B // NCORES   # fibers per core
SUB = 128          # fibers per subtile (partition dim)
GRAN = 512         # fibers per granule
NGRAN = BS // GRAN
NSUB = GRAN // SUB
NSLOT = BS // SUB  # 128 subtile slots per core
SMOOTH = 1e-6
HB = 130           # histogram bins incl. sentinel 128 (+pad to even)
LFP = 32           # roi columns incl. sentinel padding (512B DMA runs)

f32 = mybir.dt.float32
bf16 = mybir.dt.bfloat16

bfdt = ml_dtypes.bfloat16

# engine-time model (ns) used to balance the DVE/GPSIMD histogram split
_DVE_OP_NS = 196.0
_POOL_CMP_NS = 275.0


def _dve_chain_ops(m):
    return 0 if m <= 0 else 1 + max(0, (m - 3 + 1) // 2)


def _plan_pool_slots(maxlens):
    """Pick which subtile slots run their histogram on GPSIMD so that
    DVE and GPSIMD engine-busy finish together."""
    # GPSIMD cannot read per-partition scalar operands (TensorScalarPtr is
    # DVE-only), so histogram chains cannot offload to Pool. Kept as a hook.
    return set()


def _build_nc(maxlens, stage_top=False, d1=4, d2=3, xd_early=True):
    """Build the per-core program. maxlens[t] = max fiber length in subtile
    slot t (shared across cores via the round-robin deal)."""
    _register_hist_ops()
    from concourse.dve_ops import OPS as _OPS
    _h2 = next(o for o in _OPS if o.name == "HIST2_ANT")
    _h3 = next(o for o in _OPS if o.name == "HIST3F_ANT")

    pool_slots = _plan_pool_slots(maxlens)

    nc = bacc.Bacc("TRN2", target_bir_lowering=False)

    xT = nc.dram_tensor("xT", [D, BS], bf16, kind="ExternalInput")
    aug3 = nc.dram_tensor("aug3", [3, BS], bf16, kind="ExternalInput")
    rl = nc.dram_tensor("rl", [SUB, NSLOT, LFP], f32, kind="ExternalInput")
    wT2 = nc.dram_tensor("wT2", [D, K], bf16, kind="ExternalInput")
    wsq3 = nc.dram_tensor("wsq3", [3, K], bf16, kind="ExternalInput")
    tbl2 = nc.dram_tensor("tbl2", [V, 2 * K], bf16, kind="ExternalInput")
    aug2 = nc.dram_tensor("aug2", [1, 2 * K], bf16, kind="ExternalInput")
    iotav = nc.dram_tensor("iotav", [HB], bf16, kind="ExternalInput")
    ident = nc.dram_tensor("ident", [SUB, SUB], bf16, kind="ExternalInput")

    # partition-major outputs: [p, g, s, k] for 512B contiguous runs
    q_out = nc.dram_tensor("q_out", [SUB, NGRAN, NSUB, K], bf16,
                           kind="ExternalOutput")
    xd_out = nc.dram_tensor("xd_out", [SUB, NGRAN, NSUB, K], bf16,
                            kind="ExternalOutput")

    xT_v = xT[:].rearrange("(c p) n -> p c n", p=SUB)          # [128, 4, BS]

    def bcast_row(dram_ap, n):
        # DMA-read AP replicating a DRAM row across n partitions
        return bass.AP(
            tensor=dram_ap.tensor,
            offset=dram_ap.offset,
            ap=[[0, n]] + dram_ap.ap,
        )

    with tile.TileContext(nc) as tc:
        with (
            tc.tile_pool(name="consts", bufs=1) as consts,
            tc.tile_pool(name="xin", bufs=5) as xin,
            tc.tile_pool(name="rin", bufs=8) as rin,
            tc.tile_pool(name="hist", bufs=20) as hist,
            tc.tile_pool(name="histT", bufs=4) as histT,
            tc.tile_pool(name="ew_ad", bufs=d1 + 3) as ew_ad,
            tc.tile_pool(name="ew_cd", bufs=d1 + 3) as ew_cd,
            tc.tile_pool(name="ew_sh", bufs=4) as ew_sh,
            tc.tile_pool(name="ew_qn", bufs=d2 + 3) as ew_qn,
            tc.tile_pool(name="ew_xd", bufs=d1 + d2 + 3) as ew_xd,
            tc.tile_pool(name="outs", bufs=4) as outs,
            tc.tile_pool(name="psx", bufs=3, space="PSUM") as psx,
            tc.tile_pool(name="psi", bufs=2, space="PSUM") as psi,
            tc.tile_pool(name="pst", bufs=3, space="PSUM") as pst,
        ):
            # ---- constants (loaded once, SP queue; c_iov first: the
            # histogram chains need it before anything else) ----
            c_iov = consts.tile([SUB, HB], bf16)
            nc.sync.dma_start(out=c_iov, in_=bcast_row(iotav[:], SUB))
            c_id = consts.tile([SUB, SUB], bf16)
            nc.sync.dma_start(out=c_id, in_=ident[:])

            c_wT = consts.tile([SUB, 4, K], bf16)
            nc.sync.dma_start(out=c_wT, in_=wT2[:].rearrange("(c p) k -> p c k", p=SUB))
            c_wsq3 = consts.tile([3, K], bf16)
            nc.sync.dma_start(out=c_wsq3, in_=wsq3[:])
            c_tbl2 = consts.tile([V, 2 * K], bf16)
            nc.sync.dma_start(out=c_tbl2, in_=tbl2[:])
            c_aug2 = consts.tile([1, 2 * K], bf16)
            nc.sync.dma_start(out=c_aug2, in_=aug2[:])
            c_ones = consts.tile([1, SUB], bf16)
            nc.vector.memset(c_ones, 1.0)
            c_aug3 = consts.tile([3, BS], bf16)
            nc.sync.dma_start(out=c_aug3, in_=aug3[:])

            pend1 = []  # granules awaiting reciprocal + qn
            pend2 = []  # granules awaiting reduce + final normalize + DMA out

            def emit_stage1(eng=None):
                # deferred: rc never head-blocks the DVE queue
                g1, cden1, dv1, xd1 = pend1.pop(0)
                rc = ew_sh.tile([SUB, NSUB, K], f32, tag="rc")
                nc.vector.reciprocal(out=rc, in_=cden1)
                qn = ew_qn.tile([SUB, NSUB, K], f32, tag="qn")
                (eng or nc.gpsimd).tensor_tensor(
                    out=qn, in0=dv1, in1=rc, op=mybir.AluOpType.mult,
                )
                pend2.append((g1, qn, xd1))

            def emit_stage2(eng=None):
                # deferred: reduce/rn wait on long-finished qn. The row-sum
                # runs on ACT via accum_out in steady state (copy output
                # unused); in the drain it runs on the now-idle DVE.
                g2, qn2, xd2 = pend2.pop(0)
                rs = ew_sh.tile([SUB, NSUB], f32, tag="rs")
                if eng is None:
                    qsc = ew_sh.tile([SUB, NSUB, K], f32, tag="qsc")
                    for s2 in range(NSUB):
                        nc.scalar.activation(
                            out=qsc[:, s2, :], in_=qn2[:, s2, :],
                            func=mybir.ActivationFunctionType.Copy,
                            accum_out=rs[:, s2:s2 + 1],
                        )
                else:
                    nc.vector.tensor_reduce(
                        out=rs, in_=qn2,
                        axis=mybir.AxisListType.X, op=mybir.AluOpType.add,
                    )
                rn = ew_sh.tile([SUB, NSUB], f32, tag="rn")
                nc.vector.reciprocal(out=rn, in_=rs)
                qf = outs.tile([SUB, NSUB, K], bf16, tag="qf")
                rn_ap = rn[:]
                rn_b = bass.AP(
                    tensor=rn_ap.tensor, offset=rn_ap.offset,
                    ap=list(rn_ap.ap) + [[0, K]],
                )
                (eng or nc.gpsimd).tensor_tensor(
                    out=qf, in0=qn2, in1=rn_b, op=mybir.AluOpType.mult,
                )
                # outputs from the SP queue (only other outputs behind them)
                nc.sync.dma_start(out=q_out[:, g2, :, :], in_=qf[:])
                nc.sync.dma_start(
                    out=xd_out[:, g2, :, :],
                    in_=xd2[:].rearrange("p (t k) -> p t k", k=K))

            for g in range(NGRAN):
                t0 = g * NSUB  # first subtile slot of this granule

                # deferred stages of older granules first: their deps are
                # long-satisfied, so they never head-block any engine FIFO.
                if stage_top:
                    if len(pend1) >= d1:
                        emit_stage1()
                    if len(pend2) >= d2:
                        emit_stage2()

                # inputs from the ACT HWDGE queue (ACT copies ahead of them
                # complete promptly); outputs go to SP so a stalled output
                # never blocks input prefetch (DMA waits hold the SEQ).
                rt = rin.tile([SUB, NSUB, LFP], f32, tag="rt")
                nc.scalar.dma_start(out=rt, in_=rl[:, t0:t0 + NSUB, :])
                xt = xin.tile([SUB, 4, GRAN], bf16, tag="xt")
                nc.scalar.dma_start(out=xt, in_=xT_v[:, :, g * GRAN:(g + 1) * GRAN])

                psum_x = psx.tile([SUB, NSUB * K], f32, tag="px")
                psum_ad = psi.tile([SUB, NSUB, 2, K], f32, tag="pad")
                ptm = pst.tile([SUB, NSUB, SUB], bf16, tag="ptm")

                any_hist = any(maxlens[t0 + s] > 0 for s in range(NSUB))

                # x_dis matmuls first: PE work with no histogram dependency
                for s in range(NSUB):
                    for c in range(4):
                        nc.tensor.matmul(
                            psum_x[:, s * K:(s + 1) * K],
                            lhsT=xt[:, c, s * SUB:(s + 1) * SUB],
                            rhs=c_wT[:, c, :],
                            start=(c == 0), stop=False,
                        )
                    nc.tensor.matmul(
                        psum_x[:, s * K:(s + 1) * K],
                        lhsT=c_aug3[:, g * GRAN + s * SUB:g * GRAN + (s + 1) * SUB],
                        rhs=c_wsq3,
                        start=False, stop=True,
                    )

                for s in range(NSUB):
                    m = maxlens[t0 + s]
                    sc = lambda j: rt[:, s, j:j + 1]
                    # ---- per-fiber vocab histogram chain, sized to this
                    # subtile's max length; sentinel rois land in bin 128,
                    # excluded from the transpose.
                    if m > 0:
                        ha = hist.tile([SUB, HB], bf16, tag="ha")
                        hb = hist.tile([SUB, HB], bf16, tag="hb")
                        if (t0 + s) in pool_slots:
                            nc.gpsimd.tensor_scalar(
                                out=ha, in0=c_iov, scalar1=sc(0), scalar2=None,
                                op0=mybir.AluOpType.is_equal,
                            )
                            cur, nxt = ha, hb
                            for j in range(1, m):
                                nc.gpsimd.scalar_tensor_tensor(
                                    out=nxt, in0=c_iov, scalar=sc(j), in1=cur,
                                    op0=mybir.AluOpType.is_equal,
                                    op1=mybir.AluOpType.add,
                                )
                                cur, nxt = nxt, cur
                        else:
                            nc.vector._custom_dve(
                                _h3, out=ha, in0=c_iov, in1=sc(2),
                                s0=sc(0), s1=sc(1))
                            cur, nxt = ha, hb
                            for j0 in range(3, m, 2):
                                nc.vector._custom_dve(
                                    _h2, out=nxt, in0=c_iov, in1=cur,
                                    s0=sc(j0), s1=sc(j0 + 1))
                                cur, nxt = nxt, cur
                        nc.tensor.transpose(
                            out=ptm[:, s, :], in_=cur[:, 0:V], identity=c_id)

                xd = ew_xd.tile([SUB, NSUB * K], bf16, tag="xd")
                if xd_early:
                    nc.scalar.copy(out=xd, in_=psum_x)  # ACT PSUM->SBUF bf16

                if any_hist:
                    hTm = histT.tile([V, NSUB, SUB], bf16, tag="hTm")
                    nc.scalar.copy(out=hTm, in_=ptm)

                for s in range(NSUB):
                    m = maxlens[t0 + s]
                    # inter/dens: a = nF + nC - 2*inter, dens = nF + nC + s
                    if m > 0:
                        nc.tensor.matmul(
                            psum_ad[:, s, :, :], lhsT=hTm[:, s, :], rhs=c_tbl2,
                            start=True, stop=False,
                        )
                        nc.tensor.matmul(
                            psum_ad[:, s, :, :], lhsT=c_ones, rhs=c_aug2,
                            start=False, stop=True,
                        )
                    else:
                        nc.tensor.matmul(
                            psum_ad[:, s, :, :], lhsT=c_ones, rhs=c_aug2,
                            start=True, stop=True,
                        )

                # ---- elementwise on the full granule [128, 256], GPSIMD ----
                if not xd_early:
                    nc.scalar.copy(out=xd, in_=psum_x)  # ACT PSUM->SBUF bf16
                ad = ew_ad.tile([SUB, NSUB, 2, K], f32, tag="ad")
                nc.scalar.copy(out=ad, in_=psum_ad)
                a_v = ad[:, :, 0, :]
                d_v = ad[:, :, 1, :]
                xd3 = xd[:].rearrange("p (t k) -> p t k", k=K)

                t_ = ew_sh.tile([SUB, NSUB, K], f32, tag="t_")
                nc.gpsimd.tensor_tensor(
                    out=t_, in0=a_v, in1=xd3, op=mybir.AluOpType.mult,
                )
                cden = ew_cd.tile([SUB, NSUB, K], f32, tag="cden")
                nc.gpsimd.tensor_tensor(
                    out=cden, in0=t_, in1=d_v, op=mybir.AluOpType.add,
                )
                pend1.append((g, cden, d_v, xd))
                if not stage_top:
                    if len(pend1) > d1:
                        emit_stage1()
                    if len(pend2) > d2:
                        emit_stage2()

            # interleaved drain: stage2 work overlaps the remaining recips
            while pend1 or pend2:
                if pend1:
                    emit_stage1()
                if pend2:
                    emit_stage2(eng=nc.vector)

    nc.finalize()  # runs Bacc.compile(): wait-splitting, reg alloc, nop fusion
    return nc


_NC_CACHE = None
_NC_KEY = None
_LAST = None


def _get_nc(maxlens=None, **opts):
    global _NC_CACHE, _NC_KEY
    if maxlens is None:
        assert _NC_CACHE is not None
        return _NC_CACHE
    key = (tuple(int(m) for m in maxlens), tuple(sorted(opts.items())))
    if _NC_CACHE is None or _NC_KEY != key:
        _NC_CACHE = _build_nc(tuple(int(m) for m in maxlens), **opts)
        _NC_KEY = key
    return _NC_CACHE


def kernel(x, weight, fiber_rois, fiber_lens, cluster_rois, cluster_lens):
    x = np.asarray(x, np.float32)
    weight = np.asarray(weight, np.float32)
    fiber_rois = np.asarray(fiber_rois, np.int32)
    fiber_lens = np.asarray(fiber_lens, np.int32)
    cluster_rois = np.asarray(cluster_rois, np.int32)
    cluster_lens = np.asarray(cluster_lens, np.int32)

    # K-side host prep (tiny): cluster histogram table, norms, constants
    mC = (np.arange(LC)[None, :] < cluster_lens[:, None])
    histC = np.zeros((K, V), np.float32)
    for k in range(K):
        histC[k] = np.bincount(cluster_rois[k][mC[k]], minlength=V).astype(np.float32)
    nC = cluster_lens.astype(np.float32)
    # tbl2: [V, 2K]; left block 1 - 2*histC^T (-> a), right block ones (-> dens)
    tbl2 = np.concatenate(
        [1.0 - 2.0 * histC.T, np.ones((V, K), np.float32)], axis=1
    ).astype(bfdt)
    # aug2: [1, 2K]; left nC, right nC + smooth
    aug2 = np.concatenate([nC, nC + SMOOTH])[None, :].astype(bfdt)
    wsq = (weight * weight).sum(1).astype(np.float32)       # [K]
    wsq3 = np.stack([wsq, np.ones(K, np.float32), np.ones(K, np.float32)])
    wsq3 = wsq3.astype(bfdt)                                # [3, K]
    iotav = np.arange(HB).astype(bfdt)
    ident = np.eye(SUB).astype(bfdt)
    wT2 = (-2.0 * weight.T).astype(bfdt)                    # [D, K]

    # fiber-side layout prep: sort by length, deal round-robin across cores
    # so every core shares one compile-time subtile length profile; then
    # interleave slots so each granule mixes all four length quartiles.
    order = np.argsort(fiber_lens, kind="stable")
    deal = order.reshape(NSLOT, NCORES, SUB)                # [slot, core, row]
    lens_sorted = fiber_lens[order].reshape(NSLOT, NCORES * SUB)
    maxlens_sorted = lens_sorted.max(axis=1).astype(np.int64)
    # greedy-pack slots into granules so every granule carries about the
    # same DVE chain work (keeps the histogram engine evenly fed)
    chain_ops = np.array([_dve_chain_ops(int(m)) for m in maxlens_sorted])
    gran_tot = np.zeros(NGRAN)
    gran_items = [[] for _ in range(NGRAN)]
    for t in np.argsort(-chain_ops, kind="stable"):
        g = min((g for g in range(NGRAN) if len(gran_items[g]) < NSUB),
                key=lambda g: gran_tot[g])
        gran_items[g].append(t)
        gran_tot[g] += chain_ops[t]
    slot_order = np.array([t for g in range(NGRAN) for t in gran_items[g]])
    deal = deal[slot_order]
    maxlens = maxlens_sorted[slot_order]

    xsq = np.einsum("bd,bd->b", x, x).astype(np.float32)    # input norms (f32)
    xsq_hi = xsq.astype(bfdt)
    xsq_lo = (xsq - xsq_hi.astype(np.float32)).astype(bfdt)
    ones_b = np.ones(B, bfdt)
    x_bf = x.astype(bfdt)
    # rois with sentinel fold + padding columns
    rois_p = np.full((B, LFP), V, np.float32)
    rois_p[:, :LF] = np.where(np.arange(LF)[None, :] < fiber_lens[:, None],
                              fiber_rois, V).astype(np.float32)

    nc = _get_nc(maxlens)
    in_maps = []
    perms = []
    for ci in range(NCORES):
        perm = deal[:, ci, :].reshape(BS)
        perms.append(perm)
        # rl layout [p, slot, j]: fiber of slot t, partition p is perm[t*128+p]
        rl_c = rois_p[perm].reshape(NSLOT, SUB, LFP).transpose(1, 0, 2)
        in_maps.append({
            "xT": np.ascontiguousarray(x_bf[perm].T),
            "aug3": np.ascontiguousarray(
                np.stack([ones_b[perm], xsq_hi[perm], xsq_lo[perm]])),
            "rl": np.ascontiguousarray(rl_c),
            "wT2": wT2,
            "wsq3": wsq3,
            "tbl2": tbl2,
            "aug2": aug2,
            "iotav": iotav,
            "ident": ident,
        })

    res = run_bass_kernel_spmd(nc, in_maps, core_ids=list(range(NCORES)))
    global _LAST
    _LAST = res
    q = np.empty((B, K), np.float32)
    xd = np.empty((B, K), np.float32)
    for ci in range(NCORES):
        # outputs are [p, g, s, k]; fiber row of slot t=4g+s, partition p
        # is perm[t*128 + p]
        qo = res.results[ci]["q_out"].astype(np.float32)
        xo = res.results[ci]["xd_out"].astype(np.float32)
        q[perms[ci]] = qo.reshape(SUB, NSLOT, K).transpose(1, 0, 2).reshape(BS, K)
        xd[perms[ci]] = xo.reshape(SUB, NSLOT, K).transpose(1, 0, 2).reshape(BS, K)
    return (q, xd)
